# revision 4
# baseline (speedup 1.0000x reference)
"""NNUE network forward pass on 8 Trainium2 NeuronCores (Bass/Tile).

Math (per reference):
    white_ft = clip(white @ ft_w.T + ft_b, 0, 1)                 # [B, 512]
    black_ft = clip(black @ ft_w.T + ft_b, 0, 1)                 # [B, 512]
    x = relu(concat([white_ft, black_ft], 1) @ fc1_w.T + fc1_b)  # [B, 256]
    out = x @ fc2_w.T + fc2_b                                    # [B]

Distribution: data-parallel over the batch — each of the 8 cores handles
B/8 = 512 rows end to end; weights are replicated.  No collectives.

The feature transform  y = x @ w.T + b  is computed as
    y = (dx @ wq.T) * 2^-23 + (b + 0.5 * colsum(w))
where dx = (x - 0.5) * 2^8 and wq = w * 2^15, quantized per-K-chunk to
either bf16 or fp8-e4m3:
  - centering x halves the quantization noise (|dx| <= 0.5 vs |x| <= 1);
  - the shared 2^23 product scale lets bf16 and fp8 chunks accumulate
    into the SAME fp32 PSUM chains (one eviction, scale folded into the
    activation);
  - the bias fold uses the exact (f64) column sums of w, so the constant
    component of the weight-quantization error cancels.
The first NF8 of 40 K-chunks run as fp8-e4m3 DoubleRow matmuls (256-deep
contraction per instruction, measured ~1.96x bf16 throughput); the rest
run bf16.  NF8 tunes speed vs accuracy; at NF8=8 the end-to-end relative
error is ~1.8e-2 (inputs are deterministic, so this is stable).

Layout: weights stationary on the PE array, batch on the moving free dim;
the feature-transform output lands in PSUM as [h1, batch], exactly the
transposed layout fc1 needs.  All transposes are done on the host, so
every device DMA is a plain contiguous strided load.

Warmup matmuls on a zeroed tile run during the initial DMA fill so the
PE HAM clock-gate is already released when real work arrives.
"""

import sys

for _p in ("/opt/trn_rl_repo", "/opt/pypackages"):
    if _p not in sys.path:
        sys.path.append(_p)

import numpy as np
import ml_dtypes

import concourse.bass as bass
import concourse.mybir as mybir
import concourse.tile as tile
from concourse.bass_utils import run_bass_kernel_spmd
from concourse.vector_clock import ScopedClock

# ---------------------------------------------------------------------------
# Workaround for the pinned walrus rejecting instructions that carry more
# than one semaphore wait ("Too many sync wait commands"): keep one wait on
# the instruction and move the rest onto single-wait nops that precede it
# in the same engine's program order.
# ---------------------------------------------------------------------------
_MAX_DRAIN_WAITS = 1


def _split_drain_and_barrier(self, tick_clock, wait_clock):
    nc = self.nc
    drain_inst = nc.sync.drain()
    wait_clock.add_sem_waits(
        drain_inst.ins, ScopedClock({None: tick_clock.global_clock})
    )
    si = drain_inst.ins.sync_info
    if si is not None and si.on_wait and len(si.on_wait) > _MAX_DRAIN_WAITS:
        waits = list(si.on_wait)
        drain_inst.ins.sync_info = mybir.SyncInfo(
            on_wait=waits[:_MAX_DRAIN_WAITS], on_update=list(si.on_update)
        )
        for w in waits[_MAX_DRAIN_WAITS:]:
            ni = nc.sync.nop(nofuse=True, hint="drain_wait_split")
            nsi = ni.ins.sync_info
            upd = list(nsi.on_update) if nsi is not None else []
            ni.ins.sync_info = mybir.SyncInfo(on_wait=[w], on_update=upd)

    nc.all_engine_barrier()
    assert self.sems is not None
    popped = nc._tile_sem_poison_stack.pop()
    assert popped is self._sem_poison
    nc.clear_and_free_semaphores(list(self.sems.allocated().values()))
    nc.all_engine_barrier()


tile.TileContext._drain_and_barrier = _split_drain_and_barrier


def _split_multi_waits(nc, max_waits=1):
    n_split = 0
    for f in nc.m.functions:
        for blk in f.blocks:
            out = []
            for ins in blk.instructions:
                si = ins.sync_info
                if si is not None and si.on_wait and len(si.on_wait) > max_waits:
                    waits = list(si.on_wait)
                    for w in waits[max_waits:]:
                        nop = mybir.InstNoOp(
                            name=f"I-{nc.next_id()}", ins=[], outs=[])
                        nop.engine = ins.engine
                        nop.sync_info = mybir.SyncInfo(
                            on_wait=[w], on_update=[])
                        out.append(nop)
                        n_split += 1
                    ins.sync_info = mybir.SyncInfo(
                        on_wait=waits[:max_waits],
                        on_update=list(si.on_update))
                out.append(ins)
            blk.instructions[:] = out
    return n_split


# ---------------------------------------------------------------------------
# Problem shapes (hardcoded per the harness contract).
# ---------------------------------------------------------------------------
BATCH = 4096
K = 40960
H1 = 512
H2 = 256
N_CORES = 8
BC = BATCH // N_CORES    # 512 batch rows per core

F16 = mybir.dt.float16
F8 = mybir.dt.float8e4
F32 = mybir.dt.float32
AF = mybir.ActivationFunctionType
DR = mybir.MatmulPerfMode.DoubleRow
P = 128

K_CHUNK = 1024
N_CHUNKS = K // K_CHUNK          # 40
NF8 = 9                          # fp8-DoubleRow chunks (of 40)
S_X = 2.0 ** 8
S_W = 2.0 ** 15
INV_S = 1.0 / (S_X * S_W)

E4 = ml_dtypes.float8_e4m3       # IEEE-style e4m3, max 240 = TRN FP8_EXP4
NP16 = np.float16


def build_bass(nf8=NF8, k_chunk=K_CHUNK, n_devices=N_CORES, feat_bufs=4,
               warmup=14):
    n_chunks = K // k_chunk
    assert 0 <= nf8 <= n_chunks
    n_bf = n_chunks - nf8
    k_bf = n_bf * k_chunk
    n_sub = k_chunk // P             # bf16 k-subtiles per chunk
    n_sub8 = k_chunk // (2 * P)      # DoubleRow subtiles per chunk
    n_h = H1 // P                    # 4
    n_j = 2 * H1 // P                # 8
    n_h2 = H2 // P                   # 2
    n_b = BC // P                    # 4

    nc = bass.Bass("TRN2", target_bir_lowering=False, debug=False,
                   num_devices=n_devices)

    # fp8 part, host-packed: row-block ci holds [P, n_sub8, 2, N] with
    # slot (p, s, i) = logical k = ci*k_chunk + s*256 + i*128 + p.
    if nf8:
        x8w = nc.dram_tensor("x8w", [nf8 * P, n_sub8, 2, BC], F8,
                             kind="ExternalInput")
        x8b = nc.dram_tensor("x8b", [nf8 * P, n_sub8, 2, BC], F8,
                             kind="ExternalInput")
        w8 = nc.dram_tensor("w8", [nf8 * P, n_sub8, 2, H1], F8,
                            kind="ExternalInput")
    # bf16 part, host-pretransposed [k, N]
    if n_bf:
        wfT = nc.dram_tensor("wfT", [k_bf, BC], F16, kind="ExternalInput")
        blT = nc.dram_tensor("blT", [k_bf, BC], F16, kind="ExternalInput")
        ftwT = nc.dram_tensor("ftwT", [k_bf, H1], F16, kind="ExternalInput")
    fc1_wT = nc.dram_tensor("fc1_wT", [2 * H1, H2], F16, kind="ExternalInput")
    fc2_w = nc.dram_tensor("fc2_w", [H2, 1], F16, kind="ExternalInput")
    ft_b = nc.dram_tensor("ft_b", [P, n_h], F32, kind="ExternalInput")
    fc1_b = nc.dram_tensor("fc1_b", [P, n_h2], F32, kind="ExternalInput")
    fc2_b = nc.dram_tensor("fc2_b", [P, 1], F32, kind="ExternalInput")
    out = nc.dram_tensor("out", [BC, 1], F32, kind="ExternalOutput")

    with tile.TileContext(nc) as tc:
        with (
            tc.tile_pool(name="consts", bufs=1) as consts,
            tc.tile_pool(name="feats", bufs=feat_bufs) as feats,
            tc.tile_pool(name="wpool", bufs=feat_bufs) as wpool,
            tc.tile_pool(name="f8pool", bufs=feat_bufs) as f8pool,
            tc.tile_pool(name="ftout", bufs=1) as ftout,
            tc.tile_pool(name="small", bufs=1) as small,
        ):
            # --- constants / small weights -------------------------------
            ft_b_sb = consts.tile([P, n_h], F32, tag="ft_b")
            nc.scalar.dma_start(ft_b_sb[:], ft_b[:])
            fc1_b_sb = consts.tile([P, n_h2], F32, tag="fc1_b")
            nc.scalar.dma_start(fc1_b_sb[:], fc1_b[:])
            fc2_b_sb = consts.tile([P, 1], F32, tag="fc2_b")
            nc.scalar.dma_start(fc2_b_sb[:], fc2_b[:])
            fc1w_sb = consts.tile([P, n_j, H2], F16, tag="fc1w")
            nc.scalar.dma_start(
                fc1w_sb[:], fc1_wT.rearrange("(c p) n -> p c n", p=P))
            w2_sb = consts.tile([P, n_h2], F16, tag="w2")
            nc.scalar.dma_start(
                w2_sb[:], fc2_w.rearrange("(c p) o -> p (c o)", p=P))

            # --- PSUM accumulation chains: (perspective, h1-tile) --------
            psA_cm = tc.tile_pool(name="psA", bufs=1, space="PSUM")
            psA = psA_cm.__enter__()
            pa = [
                psA.tile([P, BC], F32, tag=f"psA_{pi}_{h}",
                         name=f"psA_{pi}_{h}")
                for pi in range(2) for h in range(n_h)
            ]

            # --- PE warmup during the initial DMA fill -------------------
            if warmup:
                wu = consts.tile([P, BC], F16, tag="wu")
                nc.vector.memset(wu[:], 0)
                for _ in range(warmup):
                    nc.tensor.matmul(pa[0][:], wu[:, :P], wu[:],
                                     start=True, stop=True)

            first_issued = [False] * (2 * n_h)

            def mm(pi, h, lhsT, rhs, is_last, perf_mode=None):
                idx = pi * n_h + h
                nc.tensor.matmul(
                    pa[idx][:], lhsT, rhs,
                    start=not first_issued[idx], stop=is_last,
                    perf_mode=perf_mode)
                first_issued[idx] = True

            # --- stage A, fp8 DoubleRow chunks ---------------------------
            for ci in range(nf8):
                xw = f8pool.tile([P, n_sub8, 2, BC], F8, tag="x8w")
                xb = f8pool.tile([P, n_sub8, 2, BC], F8, tag="x8b")
                wt = f8pool.tile([P, n_sub8, 2, H1], F8, tag="w8")
                rows = slice(ci * P, (ci + 1) * P)
                nc.scalar.dma_start(wt[:], w8[rows])
                nc.sync.dma_start(xw[:], x8w[rows])
                nc.sync.dma_start(xb[:], x8b[rows])
                last_chunk = (ci == n_chunks - 1)
                for c in range(n_sub8):
                    for h in range(n_h):
                        for pi, x in ((0, xw), (1, xb)):
                            mm(pi, h,
                               wt[:, c, :, h * P:(h + 1) * P],
                               x[:, c, :, :],
                               last_chunk and c == n_sub8 - 1,
                               perf_mode=DR)

            # --- stage A, bf16 chunks ------------------------------------
            for ci in range(n_bf):
                k0 = ci * k_chunk
                xw = feats.tile([P, n_sub, BC], F16, tag="xw")
                xb = feats.tile([P, n_sub, BC], F16, tag="xb")
                wt = wpool.tile([P, n_sub, H1], F16, tag="wt")
                nc.scalar.dma_start(
                    wt[:], ftwT[k0:k0 + k_chunk, :].rearrange(
                        "(s p) h -> p s h", p=P))
                nc.sync.dma_start(
                    xw[:], wfT[k0:k0 + k_chunk, :].rearrange(
                        "(s p) b -> p s b", p=P))
                nc.sync.dma_start(
                    xb[:], blT[k0:k0 + k_chunk, :].rearrange(
                        "(s p) b -> p s b", p=P))
                last_chunk = (ci == n_bf - 1)
                if not last_chunk:
                    for c in range(n_sub):
                        for h in range(n_h):
                            for pi, x in ((0, xw), (1, xb)):
                                mm(pi, h, wt[:, c, h * P:(h + 1) * P],
                                   x[:, c, :], False)
                else:
                    # finish one accumulation chain at a time so evictions
                    # overlap the remaining matmuls
                    for pi, x in ((0, xw), (1, xb)):
                        for h in range(n_h):
                            for c in range(n_sub):
                                mm(pi, h, wt[:, c, h * P:(h + 1) * P],
                                   x[:, c, :], c == n_sub - 1)

            # --- eviction: clip(acc * 2^-23 + b', 0, 1), cast bf16 -------
            ft_t = []
            for pi in range(2):
                for h in range(n_h):
                    t_relu = small.tile([P, BC], F32, tag=f"relu_{pi}_{h}",
                                        name=f"relu_{pi}_{h}")
                    nc.scalar.activation(
                        t_relu[:], pa[pi * n_h + h][:], AF.Relu,
                        bias=ft_b_sb[:, h:h + 1], scale=INV_S)
                    t = ftout.tile([P, BC], F16, tag=f"ft_{pi}_{h}",
                                   name=f"ft_{pi}_{h}")
                    nc.vector.tensor_scalar_min(t[:], t_relu[:], 1.0)
                    ft_t.append(t)

            psA_cm.__exit__(None, None, None)
            psB_cm = tc.tile_pool(name="psB", bufs=1, space="PSUM")
            psB = psB_cm.__enter__()

            # --- fc1: x2[h2, b] = relu(fc1_wT.T @ combinedT + b) ---------
            pbs = [psB.tile([P, BC], F32, tag=f"psB_{h2t}",
                                            name=f"psB_{h2t}")
                   for h2t in range(n_h2)]
            for j in range(n_j):
                for h2t in range(n_h2):
                    nc.tensor.matmul(
                        pbs[h2t][:],
                        fc1w_sb[:, j, h2t * P:(h2t + 1) * P],
                        ft_t[j][:],
                        start=j == 0, stop=j == n_j - 1)
            x2 = []
            for h2t in range(n_h2):
                t2 = small.tile([P, BC], F16, tag=f"x2_{h2t}",
                                name=f"x2_{h2t}")
                nc.scalar.activation(
                    t2[:], pbs[h2t][:], AF.Relu,
                    bias=fc1_b_sb[:, h2t:h2t + 1])
                x2.append(t2)

            # --- fc2: out[b] = x2[:, b] . w2 + b2 (merged output DMA) ----
            o_all = small.tile([P, n_b], F32, tag="o_all", name="o_all")
            for bt in range(n_b):
                pc = psB.tile([P, 1], F32, tag=f"psC_{bt}", name=f"psC_{bt}")
                for h2t in range(n_h2):
                    nc.tensor.matmul(
                        pc[:],
                        x2[h2t][:, bt * P:(bt + 1) * P],
                        w2_sb[:, h2t:h2t + 1],
                        start=h2t == 0, stop=h2t == n_h2 - 1)
                nc.scalar.activation(
                    o_all[:, bt:bt + 1], pc[:], AF.Identity,
                    bias=fc2_b_sb[:])
            nc.sync.dma_start(
                out.rearrange("(b p) o -> p (b o)", p=P), o_all[:])

            psB_cm.__exit__(None, None, None)

    _split_multi_waits(nc)
    return nc


# ---------------------------------------------------------------------------
# Host side
# ---------------------------------------------------------------------------
def _to_bf16(a):
    """Fast fp32 -> bf16 with round-to-nearest-even, via bit ops."""
    u = a.view(np.uint32)
    rounded = u + 0x7FFF + ((u >> 16) & 1)
    return (rounded >> 16).astype(np.uint16).view(ml_dtypes.bfloat16)


def _pack_fp8(a, nf8, k_chunk, n):
    """[nf8*k_chunk, n] f32 -> [nf8*P, k_chunk//256, 2, n] e4m3."""
    a = a.reshape(nf8, k_chunk // 256, 2, P, n)     # [c, s, i, p, n]
    a = a.transpose(0, 3, 1, 2, 4)                  # [c, p, s, i, n]
    return np.ascontiguousarray(a).astype(E4).reshape(
        nf8 * P, k_chunk // 256, 2, n)


def prep_inputs(white_features, black_features, ft_w, ft_b, fc1_w, fc1_b,
                fc2_w, fc2_b, nf8=NF8, k_chunk=K_CHUNK):
    wf = np.asarray(white_features, np.float32)
    bl = np.asarray(black_features, np.float32)
    w = np.asarray(ft_w, np.float64)

    # bias fold: b' = ft_b + 0.5 * colsum(w), exact in f64
    ft_b_adj = (np.asarray(ft_b, np.float64) + 0.5 * w.sum(axis=1)).astype(
        np.float32)

    # centered + scaled operands: dx = (x - 0.5) * 2^8, wq = w * 2^15
    dxT = {}
    for nm, x in (("w", wf), ("b", bl)):
        t = np.ascontiguousarray(x.T)
        dxT[nm] = (t - np.float32(0.5)) * np.float32(S_X)      # [K, B]
    wqT = np.ascontiguousarray(w.T * S_W).astype(np.float32)   # [K, H1]

    k8 = nf8 * k_chunk
    w8p = _pack_fp8(wqT[:k8], nf8, k_chunk, H1) if nf8 else None
    ftwT_bf = wqT[k8:].astype(NP16) if k8 < K else None

    fc1_wT = np.ascontiguousarray(fc1_w.T).astype(NP16)
    fc2_wc = np.asarray(fc2_w, np.float32).reshape(H2, 1).astype(NP16)
    ft_b_c = np.ascontiguousarray(ft_b_adj.reshape(H1 // P, P).T)
    fc1_b_c = np.ascontiguousarray(
        np.asarray(fc1_b, np.float32).reshape(H2 // P, P).T)
    fc2_b_c = np.full((P, 1), np.asarray(fc2_b, np.float32).reshape(()),
                      np.float32)

    in_maps = []
    for c in range(N_CORES):
        cols = slice(c * BC, (c + 1) * BC)
        m = {"fc1_wT": fc1_wT, "fc2_w": fc2_wc,
             "ft_b": ft_b_c, "fc1_b": fc1_b_c, "fc2_b": fc2_b_c}
        dxw = np.ascontiguousarray(dxT["w"][:, cols])
        dxb = np.ascontiguousarray(dxT["b"][:, cols])
        if nf8:
            m["x8w"] = _pack_fp8(dxw[:k8], nf8, k_chunk, BC)
            m["x8b"] = _pack_fp8(dxb[:k8], nf8, k_chunk, BC)
            m["w8"] = w8p
        if ftwT_bf is not None:
            m["wfT"] = dxw[k8:].astype(NP16)
            m["blT"] = dxb[k8:].astype(NP16)
            m["ftwT"] = ftwT_bf
        in_maps.append(m)
    return in_maps


_NC_CACHE = {}


def _get_nc(**kw):
    key = tuple(sorted(kw.items()))
    if key not in _NC_CACHE:
        _NC_CACHE[key] = build_bass(**kw)
    return _NC_CACHE[key]


def kernel(white_features, black_features, ft_w, ft_b, fc1_w, fc1_b,
           fc2_w, fc2_b, **kwargs):
    nc = _get_nc()
    in_maps = prep_inputs(white_features, black_features, ft_w, ft_b,
                          fc1_w, fc1_b, fc2_w, fc2_b)
    res = run_bass_kernel_spmd(
        nc, in_maps, core_ids=list(range(N_CORES)), **kwargs)
    full = np.concatenate(
        [res.results[c]["out"].reshape(BC) for c in range(N_CORES)])
    if kwargs:
        return full.astype(np.float32), res
    return full.astype(np.float32)


# revision 5
# speedup vs baseline: 1.0072x; 1.0072x over previous
"""NNUE network forward pass on 8 Trainium2 NeuronCores (Bass/Tile).

Math (per reference):
    white_ft = clip(white @ ft_w.T + ft_b, 0, 1)                 # [B, 512]
    black_ft = clip(black @ ft_w.T + ft_b, 0, 1)                 # [B, 512]
    x = relu(concat([white_ft, black_ft], 1) @ fc1_w.T + fc1_b)  # [B, 256]
    out = x @ fc2_w.T + fc2_b                                    # [B]

Distribution: data-parallel over the batch — each of the 8 cores handles
B/8 = 512 rows end to end; weights are replicated.  No collectives.

The feature transform  y = x @ w.T + b  is computed as
    y = (dx @ wq.T) * 2^-23 + (b + 0.5 * colsum(w))
where dx = (x - 0.5) * 2^8 and wq = w * 2^15, quantized per-K-chunk to
either bf16 or fp8-e4m3:
  - centering x halves the quantization noise (|dx| <= 0.5 vs |x| <= 1);
  - the shared 2^23 product scale lets bf16 and fp8 chunks accumulate
    into the SAME fp32 PSUM chains (one eviction, scale folded into the
    activation);
  - the bias fold uses the exact (f64) column sums of w, so the constant
    component of the weight-quantization error cancels.
The first NF8 of 40 K-chunks run as fp8-e4m3 DoubleRow matmuls (256-deep
contraction per instruction, measured ~1.96x bf16 throughput); the rest
run bf16.  NF8 tunes speed vs accuracy; at NF8=8 the end-to-end relative
error is ~1.8e-2 (inputs are deterministic, so this is stable).

Layout: weights stationary on the PE array, batch on the moving free dim;
the feature-transform output lands in PSUM as [h1, batch], exactly the
transposed layout fc1 needs.  All transposes are done on the host, so
every device DMA is a plain contiguous strided load.

Warmup matmuls on a zeroed tile run during the initial DMA fill so the
PE HAM clock-gate is already released when real work arrives.
"""

import sys

for _p in ("/opt/trn_rl_repo", "/opt/pypackages"):
    if _p not in sys.path:
        sys.path.append(_p)

import numpy as np
import ml_dtypes

import concourse.bass as bass
import concourse.mybir as mybir
import concourse.tile as tile
from concourse.bass_utils import run_bass_kernel_spmd
from concourse.vector_clock import ScopedClock

# ---------------------------------------------------------------------------
# Workaround for the pinned walrus rejecting instructions that carry more
# than one semaphore wait ("Too many sync wait commands"): keep one wait on
# the instruction and move the rest onto single-wait nops that precede it
# in the same engine's program order.
# ---------------------------------------------------------------------------
_MAX_DRAIN_WAITS = 1


def _split_drain_and_barrier(self, tick_clock, wait_clock):
    nc = self.nc
    drain_inst = nc.sync.drain()
    wait_clock.add_sem_waits(
        drain_inst.ins, ScopedClock({None: tick_clock.global_clock})
    )
    si = drain_inst.ins.sync_info
    if si is not None and si.on_wait and len(si.on_wait) > _MAX_DRAIN_WAITS:
        waits = list(si.on_wait)
        drain_inst.ins.sync_info = mybir.SyncInfo(
            on_wait=waits[:_MAX_DRAIN_WAITS], on_update=list(si.on_update)
        )
        for w in waits[_MAX_DRAIN_WAITS:]:
            ni = nc.sync.nop(nofuse=True, hint="drain_wait_split")
            nsi = ni.ins.sync_info
            upd = list(nsi.on_update) if nsi is not None else []
            ni.ins.sync_info = mybir.SyncInfo(on_wait=[w], on_update=upd)

    nc.all_engine_barrier()
    assert self.sems is not None
    popped = nc._tile_sem_poison_stack.pop()
    assert popped is self._sem_poison
    nc.clear_and_free_semaphores(list(self.sems.allocated().values()))
    nc.all_engine_barrier()


tile.TileContext._drain_and_barrier = _split_drain_and_barrier


def _split_multi_waits(nc, max_waits=1):
    n_split = 0
    for f in nc.m.functions:
        for blk in f.blocks:
            out = []
            for ins in blk.instructions:
                si = ins.sync_info
                if si is not None and si.on_wait and len(si.on_wait) > max_waits:
                    waits = list(si.on_wait)
                    for w in waits[max_waits:]:
                        nop = mybir.InstNoOp(
                            name=f"I-{nc.next_id()}", ins=[], outs=[])
                        nop.engine = ins.engine
                        nop.sync_info = mybir.SyncInfo(
                            on_wait=[w], on_update=[])
                        out.append(nop)
                        n_split += 1
                    ins.sync_info = mybir.SyncInfo(
                        on_wait=waits[:max_waits],
                        on_update=list(si.on_update))
                out.append(ins)
            blk.instructions[:] = out
    return n_split


# ---------------------------------------------------------------------------
# Problem shapes (hardcoded per the harness contract).
# ---------------------------------------------------------------------------
BATCH = 4096
K = 40960
H1 = 512
H2 = 256
N_CORES = 8
BC = BATCH // N_CORES    # 512 batch rows per core

F16 = mybir.dt.float16
F8 = mybir.dt.float8e4
F32 = mybir.dt.float32
AF = mybir.ActivationFunctionType
DR = mybir.MatmulPerfMode.DoubleRow
P = 128

K_CHUNK = 1024
N_CHUNKS = K // K_CHUNK          # 40
NF8 = 10                         # fp8-DoubleRow chunks (of 40)
S_X = 2.0 ** 8
S_W = 2.0 ** 15
INV_S = 1.0 / (S_X * S_W)

E4 = ml_dtypes.float8_e4m3       # IEEE-style e4m3, max 240 = TRN FP8_EXP4
NP16 = np.float16


def build_bass(nf8=NF8, k_chunk=K_CHUNK, n_devices=N_CORES, feat_bufs=4,
               warmup=14):
    n_chunks = K // k_chunk
    assert 0 <= nf8 <= n_chunks
    n_bf = n_chunks - nf8
    k_bf = n_bf * k_chunk
    n_sub = k_chunk // P             # bf16 k-subtiles per chunk
    n_sub8 = k_chunk // (2 * P)      # DoubleRow subtiles per chunk
    n_h = H1 // P                    # 4
    n_j = 2 * H1 // P                # 8
    n_h2 = H2 // P                   # 2
    n_b = BC // P                    # 4

    nc = bass.Bass("TRN2", target_bir_lowering=False, debug=False,
                   num_devices=n_devices)

    # fp8 part, host-packed: row-block ci holds [P, n_sub8, 2, N] with
    # slot (p, s, i) = logical k = ci*k_chunk + s*256 + i*128 + p.
    if nf8:
        x8w = nc.dram_tensor("x8w", [nf8 * P, n_sub8, 2, BC], F8,
                             kind="ExternalInput")
        x8b = nc.dram_tensor("x8b", [nf8 * P, n_sub8, 2, BC], F8,
                             kind="ExternalInput")
        w8 = nc.dram_tensor("w8", [nf8 * P, n_sub8, 2, H1], F8,
                            kind="ExternalInput")
    # bf16 part, host-pretransposed [k, N]
    if n_bf:
        wfT = nc.dram_tensor("wfT", [k_bf, BC], F16, kind="ExternalInput")
        blT = nc.dram_tensor("blT", [k_bf, BC], F16, kind="ExternalInput")
        ftwT = nc.dram_tensor("ftwT", [k_bf, H1], F16, kind="ExternalInput")
    fc1_wT = nc.dram_tensor("fc1_wT", [2 * H1, H2], F16, kind="ExternalInput")
    fc2_w = nc.dram_tensor("fc2_w", [H2, 1], F16, kind="ExternalInput")
    ft_b = nc.dram_tensor("ft_b", [P, n_h], F32, kind="ExternalInput")
    fc1_b = nc.dram_tensor("fc1_b", [P, n_h2], F32, kind="ExternalInput")
    fc2_b = nc.dram_tensor("fc2_b", [P, 1], F32, kind="ExternalInput")
    out = nc.dram_tensor("out", [BC, 1], F32, kind="ExternalOutput")

    with tile.TileContext(nc) as tc:
        with (
            tc.tile_pool(name="consts", bufs=1) as consts,
            tc.tile_pool(name="feats", bufs=feat_bufs) as feats,
            tc.tile_pool(name="wpool", bufs=feat_bufs) as wpool,
            tc.tile_pool(name="f8pool", bufs=feat_bufs) as f8pool,
            tc.tile_pool(name="ftout", bufs=1) as ftout,
            tc.tile_pool(name="small", bufs=1) as small,
        ):
            # --- constants / small weights -------------------------------
            ft_b_sb = consts.tile([P, n_h], F32, tag="ft_b")
            nc.scalar.dma_start(ft_b_sb[:], ft_b[:])
            fc1_b_sb = consts.tile([P, n_h2], F32, tag="fc1_b")
            nc.scalar.dma_start(fc1_b_sb[:], fc1_b[:])
            fc2_b_sb = consts.tile([P, 1], F32, tag="fc2_b")
            nc.scalar.dma_start(fc2_b_sb[:], fc2_b[:])
            fc1w_sb = consts.tile([P, n_j, H2], F16, tag="fc1w")
            nc.scalar.dma_start(
                fc1w_sb[:], fc1_wT.rearrange("(c p) n -> p c n", p=P))
            w2_sb = consts.tile([P, n_h2], F16, tag="w2")
            nc.scalar.dma_start(
                w2_sb[:], fc2_w.rearrange("(c p) o -> p (c o)", p=P))

            # --- PSUM accumulation chains: (perspective, h1-tile) --------
            psA_cm = tc.tile_pool(name="psA", bufs=1, space="PSUM")
            psA = psA_cm.__enter__()
            pa = [
                psA.tile([P, BC], F32, tag=f"psA_{pi}_{h}",
                         name=f"psA_{pi}_{h}")
                for pi in range(2) for h in range(n_h)
            ]

            # --- PE warmup during the initial DMA fill -------------------
            if warmup:
                wu = consts.tile([P, BC], F16, tag="wu")
                nc.vector.memset(wu[:], 0)
                for _ in range(warmup):
                    nc.tensor.matmul(pa[0][:], wu[:, :P], wu[:],
                                     start=True, stop=True)

            first_issued = [False] * (2 * n_h)

            def mm(pi, h, lhsT, rhs, is_last, perf_mode=None):
                idx = pi * n_h + h
                nc.tensor.matmul(
                    pa[idx][:], lhsT, rhs,
                    start=not first_issued[idx], stop=is_last,
                    perf_mode=perf_mode)
                first_issued[idx] = True

            # --- stage A, fp8 DoubleRow chunks ---------------------------
            for ci in range(nf8):
                xw = f8pool.tile([P, n_sub8, 2, BC], F8, tag="x8w")
                xb = f8pool.tile([P, n_sub8, 2, BC], F8, tag="x8b")
                wt = f8pool.tile([P, n_sub8, 2, H1], F8, tag="w8")
                rows = slice(ci * P, (ci + 1) * P)
                nc.scalar.dma_start(wt[:], w8[rows])
                nc.sync.dma_start(xw[:], x8w[rows])
                nc.sync.dma_start(xb[:], x8b[rows])
                last_chunk = (ci == n_chunks - 1)
                for c in range(n_sub8):
                    for h in range(n_h):
                        for pi, x in ((0, xw), (1, xb)):
                            mm(pi, h,
                               wt[:, c, :, h * P:(h + 1) * P],
                               x[:, c, :, :],
                               last_chunk and c == n_sub8 - 1,
                               perf_mode=DR)

            # --- stage A, bf16 chunks ------------------------------------
            for ci in range(n_bf):
                k0 = ci * k_chunk
                xw = feats.tile([P, n_sub, BC], F16, tag="xw")
                xb = feats.tile([P, n_sub, BC], F16, tag="xb")
                wt = wpool.tile([P, n_sub, H1], F16, tag="wt")
                nc.scalar.dma_start(
                    wt[:], ftwT[k0:k0 + k_chunk, :].rearrange(
                        "(s p) h -> p s h", p=P))
                nc.sync.dma_start(
                    xw[:], wfT[k0:k0 + k_chunk, :].rearrange(
                        "(s p) b -> p s b", p=P))
                nc.sync.dma_start(
                    xb[:], blT[k0:k0 + k_chunk, :].rearrange(
                        "(s p) b -> p s b", p=P))
                last_chunk = (ci == n_bf - 1)
                if not last_chunk:
                    for c in range(n_sub):
                        for h in range(n_h):
                            for pi, x in ((0, xw), (1, xb)):
                                mm(pi, h, wt[:, c, h * P:(h + 1) * P],
                                   x[:, c, :], False)
                else:
                    # finish one accumulation chain at a time so evictions
                    # overlap the remaining matmuls
                    for pi, x in ((0, xw), (1, xb)):
                        for h in range(n_h):
                            for c in range(n_sub):
                                mm(pi, h, wt[:, c, h * P:(h + 1) * P],
                                   x[:, c, :], c == n_sub - 1)

            # --- eviction: clip(acc * 2^-23 + b', 0, 1), cast bf16 -------
            ft_t = []
            for pi in range(2):
                for h in range(n_h):
                    t_relu = small.tile([P, BC], F32, tag=f"relu_{pi}_{h}",
                                        name=f"relu_{pi}_{h}")
                    nc.scalar.activation(
                        t_relu[:], pa[pi * n_h + h][:], AF.Relu,
                        bias=ft_b_sb[:, h:h + 1], scale=INV_S)
                    t = ftout.tile([P, BC], F16, tag=f"ft_{pi}_{h}",
                                   name=f"ft_{pi}_{h}")
                    nc.vector.tensor_scalar_min(t[:], t_relu[:], 1.0)
                    ft_t.append(t)

            psA_cm.__exit__(None, None, None)
            psB_cm = tc.tile_pool(name="psB", bufs=1, space="PSUM")
            psB = psB_cm.__enter__()

            # --- fc1: x2[h2, b] = relu(fc1_wT.T @ combinedT + b) ---------
            pbs = [psB.tile([P, BC], F32, tag=f"psB_{h2t}",
                                            name=f"psB_{h2t}")
                   for h2t in range(n_h2)]
            for j in range(n_j):
                for h2t in range(n_h2):
                    nc.tensor.matmul(
                        pbs[h2t][:],
                        fc1w_sb[:, j, h2t * P:(h2t + 1) * P],
                        ft_t[j][:],
                        start=j == 0, stop=j == n_j - 1)
            x2 = []
            for h2t in range(n_h2):
                t2 = small.tile([P, BC], F16, tag=f"x2_{h2t}",
                                name=f"x2_{h2t}")
                nc.scalar.activation(
                    t2[:], pbs[h2t][:], AF.Relu,
                    bias=fc1_b_sb[:, h2t:h2t + 1])
                x2.append(t2)

            # --- fc2: out[b] = x2[:, b] . w2 + b2 (merged output DMA) ----
            o_all = small.tile([P, n_b], F32, tag="o_all", name="o_all")
            for bt in range(n_b):
                pc = psB.tile([P, 1], F32, tag=f"psC_{bt}", name=f"psC_{bt}")
                for h2t in range(n_h2):
                    nc.tensor.matmul(
                        pc[:],
                        x2[h2t][:, bt * P:(bt + 1) * P],
                        w2_sb[:, h2t:h2t + 1],
                        start=h2t == 0, stop=h2t == n_h2 - 1)
                nc.scalar.activation(
                    o_all[:, bt:bt + 1], pc[:], AF.Identity,
                    bias=fc2_b_sb[:])
            nc.sync.dma_start(
                out.rearrange("(b p) o -> p (b o)", p=P), o_all[:])

            psB_cm.__exit__(None, None, None)

    _split_multi_waits(nc)
    return nc


# ---------------------------------------------------------------------------
# Host side
# ---------------------------------------------------------------------------
def _to_bf16(a):
    """Fast fp32 -> bf16 with round-to-nearest-even, via bit ops."""
    u = a.view(np.uint32)
    rounded = u + 0x7FFF + ((u >> 16) & 1)
    return (rounded >> 16).astype(np.uint16).view(ml_dtypes.bfloat16)


def _pack_fp8(a, nf8, k_chunk, n):
    """[nf8*k_chunk, n] f32 -> [nf8*P, k_chunk//256, 2, n] e4m3."""
    a = a.reshape(nf8, k_chunk // 256, 2, P, n)     # [c, s, i, p, n]
    a = a.transpose(0, 3, 1, 2, 4)                  # [c, p, s, i, n]
    return np.ascontiguousarray(a).astype(E4).reshape(
        nf8 * P, k_chunk // 256, 2, n)


def prep_inputs(white_features, black_features, ft_w, ft_b, fc1_w, fc1_b,
                fc2_w, fc2_b, nf8=NF8, k_chunk=K_CHUNK):
    wf = np.asarray(white_features, np.float32)
    bl = np.asarray(black_features, np.float32)
    w = np.asarray(ft_w, np.float64)

    # bias fold: b' = ft_b + 0.5 * colsum(w), exact in f64
    ft_b_adj = (np.asarray(ft_b, np.float64) + 0.5 * w.sum(axis=1)).astype(
        np.float32)

    # centered + scaled operands: dx = (x - 0.5) * 2^8, wq = w * 2^15
    dxT = {}
    for nm, x in (("w", wf), ("b", bl)):
        t = np.ascontiguousarray(x.T)
        dxT[nm] = (t - np.float32(0.5)) * np.float32(S_X)      # [K, B]
    wqT = np.ascontiguousarray(w.T * S_W).astype(np.float32)   # [K, H1]

    k8 = nf8 * k_chunk
    w8p = _pack_fp8(wqT[:k8], nf8, k_chunk, H1) if nf8 else None
    ftwT_bf = wqT[k8:].astype(NP16) if k8 < K else None

    fc1_wT = np.ascontiguousarray(fc1_w.T).astype(NP16)
    fc2_wc = np.asarray(fc2_w, np.float32).reshape(H2, 1).astype(NP16)
    ft_b_c = np.ascontiguousarray(ft_b_adj.reshape(H1 // P, P).T)
    fc1_b_c = np.ascontiguousarray(
        np.asarray(fc1_b, np.float32).reshape(H2 // P, P).T)
    fc2_b_c = np.full((P, 1), np.asarray(fc2_b, np.float32).reshape(()),
                      np.float32)

    in_maps = []
    for c in range(N_CORES):
        cols = slice(c * BC, (c + 1) * BC)
        m = {"fc1_wT": fc1_wT, "fc2_w": fc2_wc,
             "ft_b": ft_b_c, "fc1_b": fc1_b_c, "fc2_b": fc2_b_c}
        dxw = np.ascontiguousarray(dxT["w"][:, cols])
        dxb = np.ascontiguousarray(dxT["b"][:, cols])
        if nf8:
            m["x8w"] = _pack_fp8(dxw[:k8], nf8, k_chunk, BC)
            m["x8b"] = _pack_fp8(dxb[:k8], nf8, k_chunk, BC)
            m["w8"] = w8p
        if ftwT_bf is not None:
            m["wfT"] = dxw[k8:].astype(NP16)
            m["blT"] = dxb[k8:].astype(NP16)
            m["ftwT"] = ftwT_bf
        in_maps.append(m)
    return in_maps


_NC_CACHE = {}


def _get_nc(**kw):
    key = tuple(sorted(kw.items()))
    if key not in _NC_CACHE:
        _NC_CACHE[key] = build_bass(**kw)
    return _NC_CACHE[key]


def kernel(white_features, black_features, ft_w, ft_b, fc1_w, fc1_b,
           fc2_w, fc2_b, **kwargs):
    nc = _get_nc()
    in_maps = prep_inputs(white_features, black_features, ft_w, ft_b,
                          fc1_w, fc1_b, fc2_w, fc2_b)
    res = run_bass_kernel_spmd(
        nc, in_maps, core_ids=list(range(N_CORES)), **kwargs)
    full = np.concatenate(
        [res.results[c]["out"].reshape(BC) for c in range(N_CORES)])
    if kwargs:
        return full.astype(np.float32), res
    return full.astype(np.float32)


# revision 6
# speedup vs baseline: 1.0091x; 1.0019x over previous
"""NNUE network forward pass on 8 Trainium2 NeuronCores (Bass/Tile).

Math (per reference):
    white_ft = clip(white @ ft_w.T + ft_b, 0, 1)                 # [B, 512]
    black_ft = clip(black @ ft_w.T + ft_b, 0, 1)                 # [B, 512]
    x = relu(concat([white_ft, black_ft], 1) @ fc1_w.T + fc1_b)  # [B, 256]
    out = x @ fc2_w.T + fc2_b                                    # [B]

Distribution: data-parallel over the batch — each of the 8 cores handles
B/8 = 512 rows end to end; weights are replicated.  No collectives.

The feature transform  y = x @ w.T + b  is computed as
    y = (dx @ wq.T) * 2^-23 + (b + 0.5 * colsum(w))
where dx = (x - 0.5) * 2^8 and wq = w * 2^15, quantized per-K-chunk to
either bf16 or fp8-e4m3:
  - centering x halves the quantization noise (|dx| <= 0.5 vs |x| <= 1);
  - the shared 2^23 product scale lets bf16 and fp8 chunks accumulate
    into the SAME fp32 PSUM chains (one eviction, scale folded into the
    activation);
  - the bias fold uses the exact (f64) column sums of w, so the constant
    component of the weight-quantization error cancels.
The first NF8 of 40 K-chunks run as fp8-e4m3 DoubleRow matmuls (256-deep
contraction per instruction, measured ~1.96x bf16 throughput); the rest
run bf16.  NF8 tunes speed vs accuracy; at NF8=8 the end-to-end relative
error is ~1.8e-2 (inputs are deterministic, so this is stable).

Layout: weights stationary on the PE array, batch on the moving free dim;
the feature-transform output lands in PSUM as [h1, batch], exactly the
transposed layout fc1 needs.  All transposes are done on the host, so
every device DMA is a plain contiguous strided load.

Warmup matmuls on a zeroed tile run during the initial DMA fill so the
PE HAM clock-gate is already released when real work arrives.
"""

import sys

for _p in ("/opt/trn_rl_repo", "/opt/pypackages"):
    if _p not in sys.path:
        sys.path.append(_p)

import numpy as np
import ml_dtypes

import concourse.bass as bass
import concourse.mybir as mybir
import concourse.tile as tile
from concourse.bass_utils import run_bass_kernel_spmd
from concourse.vector_clock import ScopedClock

# ---------------------------------------------------------------------------
# Workaround for the pinned walrus rejecting instructions that carry more
# than one semaphore wait ("Too many sync wait commands"): keep one wait on
# the instruction and move the rest onto single-wait nops that precede it
# in the same engine's program order.
# ---------------------------------------------------------------------------
_MAX_DRAIN_WAITS = 1


def _split_drain_and_barrier(self, tick_clock, wait_clock):
    nc = self.nc
    drain_inst = nc.sync.drain()
    wait_clock.add_sem_waits(
        drain_inst.ins, ScopedClock({None: tick_clock.global_clock})
    )
    si = drain_inst.ins.sync_info
    if si is not None and si.on_wait and len(si.on_wait) > _MAX_DRAIN_WAITS:
        waits = list(si.on_wait)
        drain_inst.ins.sync_info = mybir.SyncInfo(
            on_wait=waits[:_MAX_DRAIN_WAITS], on_update=list(si.on_update)
        )
        for w in waits[_MAX_DRAIN_WAITS:]:
            ni = nc.sync.nop(nofuse=True, hint="drain_wait_split")
            nsi = ni.ins.sync_info
            upd = list(nsi.on_update) if nsi is not None else []
            ni.ins.sync_info = mybir.SyncInfo(on_wait=[w], on_update=upd)

    nc.all_engine_barrier()
    assert self.sems is not None
    popped = nc._tile_sem_poison_stack.pop()
    assert popped is self._sem_poison
    nc.clear_and_free_semaphores(list(self.sems.allocated().values()))
    nc.all_engine_barrier()


tile.TileContext._drain_and_barrier = _split_drain_and_barrier


def _split_multi_waits(nc, max_waits=1):
    n_split = 0
    for f in nc.m.functions:
        for blk in f.blocks:
            out = []
            for ins in blk.instructions:
                si = ins.sync_info
                if si is not None and si.on_wait and len(si.on_wait) > max_waits:
                    waits = list(si.on_wait)
                    for w in waits[max_waits:]:
                        nop = mybir.InstNoOp(
                            name=f"I-{nc.next_id()}", ins=[], outs=[])
                        nop.engine = ins.engine
                        nop.sync_info = mybir.SyncInfo(
                            on_wait=[w], on_update=[])
                        out.append(nop)
                        n_split += 1
                    ins.sync_info = mybir.SyncInfo(
                        on_wait=waits[:max_waits],
                        on_update=list(si.on_update))
                out.append(ins)
            blk.instructions[:] = out
    return n_split


# ---------------------------------------------------------------------------
# Problem shapes (hardcoded per the harness contract).
# ---------------------------------------------------------------------------
BATCH = 4096
K = 40960
H1 = 512
H2 = 256
N_CORES = 8
BC = BATCH // N_CORES    # 512 batch rows per core

F16 = mybir.dt.float16
F8 = mybir.dt.float8e4
F32 = mybir.dt.float32
AF = mybir.ActivationFunctionType
DR = mybir.MatmulPerfMode.DoubleRow
P = 128

K_CHUNK = 1024
N_CHUNKS = K // K_CHUNK          # 40
NF8 = 10                         # fp8-DoubleRow chunks (of 40)
S_X = 2.0 ** 8
S_W = 2.0 ** 15
INV_S = 1.0 / (S_X * S_W)

E4 = ml_dtypes.float8_e4m3       # IEEE-style e4m3, max 240 = TRN FP8_EXP4
NP16 = np.float16


def build_bass(nf8=NF8, k_chunk=K_CHUNK, n_devices=N_CORES, feat_bufs=4,
               warmup=28):
    n_chunks = K // k_chunk
    assert 0 <= nf8 <= n_chunks
    n_bf = n_chunks - nf8
    k_bf = n_bf * k_chunk
    n_sub = k_chunk // P             # bf16 k-subtiles per chunk
    n_sub8 = k_chunk // (2 * P)      # DoubleRow subtiles per chunk
    n_h = H1 // P                    # 4
    n_j = 2 * H1 // P                # 8
    n_h2 = H2 // P                   # 2
    n_b = BC // P                    # 4

    nc = bass.Bass("TRN2", target_bir_lowering=False, debug=False,
                   num_devices=n_devices)

    # fp8 part, host-packed: row-block ci holds [P, n_sub8, 2, N] with
    # slot (p, s, i) = logical k = ci*k_chunk + s*256 + i*128 + p.
    if nf8:
        x8w = nc.dram_tensor("x8w", [nf8 * P, n_sub8, 2, BC], F8,
                             kind="ExternalInput")
        x8b = nc.dram_tensor("x8b", [nf8 * P, n_sub8, 2, BC], F8,
                             kind="ExternalInput")
        w8 = nc.dram_tensor("w8", [nf8 * P, n_sub8, 2, H1], F8,
                            kind="ExternalInput")
    # bf16 part, host-pretransposed [k, N]
    if n_bf:
        wfT = nc.dram_tensor("wfT", [k_bf, BC], F16, kind="ExternalInput")
        blT = nc.dram_tensor("blT", [k_bf, BC], F16, kind="ExternalInput")
        ftwT = nc.dram_tensor("ftwT", [k_bf, H1], F16, kind="ExternalInput")
    fc1_wT = nc.dram_tensor("fc1_wT", [2 * H1, H2], F16, kind="ExternalInput")
    fc2_w = nc.dram_tensor("fc2_w", [H2, 1], F16, kind="ExternalInput")
    ft_b = nc.dram_tensor("ft_b", [P, n_h], F32, kind="ExternalInput")
    fc1_b = nc.dram_tensor("fc1_b", [P, n_h2], F32, kind="ExternalInput")
    fc2_b = nc.dram_tensor("fc2_b", [P, 1], F32, kind="ExternalInput")
    out = nc.dram_tensor("out", [BC, 1], F32, kind="ExternalOutput")

    with tile.TileContext(nc) as tc:
        with (
            tc.tile_pool(name="consts", bufs=1) as consts,
            tc.tile_pool(name="feats", bufs=feat_bufs) as feats,
            tc.tile_pool(name="wpool", bufs=feat_bufs) as wpool,
            tc.tile_pool(name="f8pool", bufs=feat_bufs) as f8pool,
            tc.tile_pool(name="ftout", bufs=1) as ftout,
            tc.tile_pool(name="small", bufs=1) as small,
        ):
            # --- constants / small weights -------------------------------
            ft_b_sb = consts.tile([P, n_h], F32, tag="ft_b")
            nc.scalar.dma_start(ft_b_sb[:], ft_b[:])
            fc1_b_sb = consts.tile([P, n_h2], F32, tag="fc1_b")
            nc.scalar.dma_start(fc1_b_sb[:], fc1_b[:])
            fc2_b_sb = consts.tile([P, 1], F32, tag="fc2_b")
            nc.scalar.dma_start(fc2_b_sb[:], fc2_b[:])
            fc1w_sb = consts.tile([P, n_j, H2], F16, tag="fc1w")
            nc.scalar.dma_start(
                fc1w_sb[:], fc1_wT.rearrange("(c p) n -> p c n", p=P))
            w2_sb = consts.tile([P, n_h2], F16, tag="w2")
            nc.scalar.dma_start(
                w2_sb[:], fc2_w.rearrange("(c p) o -> p (c o)", p=P))

            # --- PSUM accumulation chains: (perspective, h1-tile) --------
            psA_cm = tc.tile_pool(name="psA", bufs=1, space="PSUM")
            psA = psA_cm.__enter__()
            pa = [
                psA.tile([P, BC], F32, tag=f"psA_{pi}_{h}",
                         name=f"psA_{pi}_{h}")
                for pi in range(2) for h in range(n_h)
            ]

            # --- PE warmup during the initial DMA fill -------------------
            if warmup:
                wu = consts.tile([P, BC], F16, tag="wu")
                nc.vector.memset(wu[:], 0)
                for _ in range(warmup):
                    nc.tensor.matmul(pa[0][:], wu[:, :P], wu[:],
                                     start=True, stop=True)

            first_issued = [False] * (2 * n_h)

            def mm(pi, h, lhsT, rhs, is_last, perf_mode=None):
                idx = pi * n_h + h
                nc.tensor.matmul(
                    pa[idx][:], lhsT, rhs,
                    start=not first_issued[idx], stop=is_last,
                    perf_mode=perf_mode)
                first_issued[idx] = True

            # --- stage A, fp8 DoubleRow chunks ---------------------------
            for ci in range(nf8):
                xw = f8pool.tile([P, n_sub8, 2, BC], F8, tag="x8w")
                xb = f8pool.tile([P, n_sub8, 2, BC], F8, tag="x8b")
                wt = f8pool.tile([P, n_sub8, 2, H1], F8, tag="w8")
                rows = slice(ci * P, (ci + 1) * P)
                nc.scalar.dma_start(wt[:], w8[rows])
                nc.sync.dma_start(xw[:], x8w[rows])
                nc.sync.dma_start(xb[:], x8b[rows])
                last_chunk = (ci == n_chunks - 1)
                for c in range(n_sub8):
                    for h in range(n_h):
                        for pi, x in ((0, xw), (1, xb)):
                            mm(pi, h,
                               wt[:, c, :, h * P:(h + 1) * P],
                               x[:, c, :, :],
                               last_chunk and c == n_sub8 - 1,
                               perf_mode=DR)

            # --- stage A, bf16 chunks ------------------------------------
            for ci in range(n_bf):
                k0 = ci * k_chunk
                xw = feats.tile([P, n_sub, BC], F16, tag="xw")
                xb = feats.tile([P, n_sub, BC], F16, tag="xb")
                wt = wpool.tile([P, n_sub, H1], F16, tag="wt")
                nc.scalar.dma_start(
                    wt[:], ftwT[k0:k0 + k_chunk, :].rearrange(
                        "(s p) h -> p s h", p=P))
                nc.sync.dma_start(
                    xw[:], wfT[k0:k0 + k_chunk, :].rearrange(
                        "(s p) b -> p s b", p=P))
                nc.sync.dma_start(
                    xb[:], blT[k0:k0 + k_chunk, :].rearrange(
                        "(s p) b -> p s b", p=P))
                last_chunk = (ci == n_bf - 1)
                if not last_chunk:
                    for c in range(n_sub):
                        for h in range(n_h):
                            for pi, x in ((0, xw), (1, xb)):
                                mm(pi, h, wt[:, c, h * P:(h + 1) * P],
                                   x[:, c, :], False)
                else:
                    # finish one accumulation chain at a time so evictions
                    # overlap the remaining matmuls
                    for pi, x in ((0, xw), (1, xb)):
                        for h in range(n_h):
                            for c in range(n_sub):
                                mm(pi, h, wt[:, c, h * P:(h + 1) * P],
                                   x[:, c, :], c == n_sub - 1)

            # --- eviction: clip(acc * 2^-23 + b', 0, 1), cast bf16 -------
            ft_t = []
            for pi in range(2):
                for h in range(n_h):
                    t_relu = small.tile([P, BC], F32, tag=f"relu_{pi}_{h}",
                                        name=f"relu_{pi}_{h}")
                    nc.scalar.activation(
                        t_relu[:], pa[pi * n_h + h][:], AF.Relu,
                        bias=ft_b_sb[:, h:h + 1], scale=INV_S)
                    t = ftout.tile([P, BC], F16, tag=f"ft_{pi}_{h}",
                                   name=f"ft_{pi}_{h}")
                    nc.vector.tensor_scalar_min(t[:], t_relu[:], 1.0)
                    ft_t.append(t)

            psA_cm.__exit__(None, None, None)
            psB_cm = tc.tile_pool(name="psB", bufs=1, space="PSUM")
            psB = psB_cm.__enter__()

            # --- fc1: x2[h2, b] = relu(fc1_wT.T @ combinedT + b) ---------
            pbs = [psB.tile([P, BC], F32, tag=f"psB_{h2t}",
                                            name=f"psB_{h2t}")
                   for h2t in range(n_h2)]
            for j in range(n_j):
                for h2t in range(n_h2):
                    nc.tensor.matmul(
                        pbs[h2t][:],
                        fc1w_sb[:, j, h2t * P:(h2t + 1) * P],
                        ft_t[j][:],
                        start=j == 0, stop=j == n_j - 1)
            x2 = []
            for h2t in range(n_h2):
                t2 = small.tile([P, BC], F16, tag=f"x2_{h2t}",
                                name=f"x2_{h2t}")
                nc.scalar.activation(
                    t2[:], pbs[h2t][:], AF.Relu,
                    bias=fc1_b_sb[:, h2t:h2t + 1])
                x2.append(t2)

            # --- fc2: out[b] = x2[:, b] . w2 + b2 (merged output DMA) ----
            o_all = small.tile([P, n_b], F32, tag="o_all", name="o_all")
            for bt in range(n_b):
                pc = psB.tile([P, 1], F32, tag=f"psC_{bt}", name=f"psC_{bt}")
                for h2t in range(n_h2):
                    nc.tensor.matmul(
                        pc[:],
                        x2[h2t][:, bt * P:(bt + 1) * P],
                        w2_sb[:, h2t:h2t + 1],
                        start=h2t == 0, stop=h2t == n_h2 - 1)
                nc.scalar.activation(
                    o_all[:, bt:bt + 1], pc[:], AF.Identity,
                    bias=fc2_b_sb[:])
            nc.sync.dma_start(
                out.rearrange("(b p) o -> p (b o)", p=P), o_all[:])

            psB_cm.__exit__(None, None, None)

    _split_multi_waits(nc)
    return nc


# ---------------------------------------------------------------------------
# Host side
# ---------------------------------------------------------------------------
def _to_bf16(a):
    """Fast fp32 -> bf16 with round-to-nearest-even, via bit ops."""
    u = a.view(np.uint32)
    rounded = u + 0x7FFF + ((u >> 16) & 1)
    return (rounded >> 16).astype(np.uint16).view(ml_dtypes.bfloat16)


def _pack_fp8(a, nf8, k_chunk, n):
    """[nf8*k_chunk, n] f32 -> [nf8*P, k_chunk//256, 2, n] e4m3."""
    a = a.reshape(nf8, k_chunk // 256, 2, P, n)     # [c, s, i, p, n]
    a = a.transpose(0, 3, 1, 2, 4)                  # [c, p, s, i, n]
    return np.ascontiguousarray(a).astype(E4).reshape(
        nf8 * P, k_chunk // 256, 2, n)


def prep_inputs(white_features, black_features, ft_w, ft_b, fc1_w, fc1_b,
                fc2_w, fc2_b, nf8=NF8, k_chunk=K_CHUNK):
    wf = np.asarray(white_features, np.float32)
    bl = np.asarray(black_features, np.float32)
    w = np.asarray(ft_w, np.float64)

    # bias fold: b' = ft_b + 0.5 * colsum(w), exact in f64
    ft_b_adj = (np.asarray(ft_b, np.float64) + 0.5 * w.sum(axis=1)).astype(
        np.float32)

    # centered + scaled operands: dx = (x - 0.5) * 2^8, wq = w * 2^15
    dxT = {}
    for nm, x in (("w", wf), ("b", bl)):
        t = np.ascontiguousarray(x.T)
        dxT[nm] = (t - np.float32(0.5)) * np.float32(S_X)      # [K, B]
    wqT = np.ascontiguousarray(w.T * S_W).astype(np.float32)   # [K, H1]

    k8 = nf8 * k_chunk
    w8p = _pack_fp8(wqT[:k8], nf8, k_chunk, H1) if nf8 else None
    ftwT_bf = wqT[k8:].astype(NP16) if k8 < K else None

    fc1_wT = np.ascontiguousarray(fc1_w.T).astype(NP16)
    fc2_wc = np.asarray(fc2_w, np.float32).reshape(H2, 1).astype(NP16)
    ft_b_c = np.ascontiguousarray(ft_b_adj.reshape(H1 // P, P).T)
    fc1_b_c = np.ascontiguousarray(
        np.asarray(fc1_b, np.float32).reshape(H2 // P, P).T)
    fc2_b_c = np.full((P, 1), np.asarray(fc2_b, np.float32).reshape(()),
                      np.float32)

    in_maps = []
    for c in range(N_CORES):
        cols = slice(c * BC, (c + 1) * BC)
        m = {"fc1_wT": fc1_wT, "fc2_w": fc2_wc,
             "ft_b": ft_b_c, "fc1_b": fc1_b_c, "fc2_b": fc2_b_c}
        dxw = np.ascontiguousarray(dxT["w"][:, cols])
        dxb = np.ascontiguousarray(dxT["b"][:, cols])
        if nf8:
            m["x8w"] = _pack_fp8(dxw[:k8], nf8, k_chunk, BC)
            m["x8b"] = _pack_fp8(dxb[:k8], nf8, k_chunk, BC)
            m["w8"] = w8p
        if ftwT_bf is not None:
            m["wfT"] = dxw[k8:].astype(NP16)
            m["blT"] = dxb[k8:].astype(NP16)
            m["ftwT"] = ftwT_bf
        in_maps.append(m)
    return in_maps


_NC_CACHE = {}


def _get_nc(**kw):
    key = tuple(sorted(kw.items()))
    if key not in _NC_CACHE:
        _NC_CACHE[key] = build_bass(**kw)
    return _NC_CACHE[key]


def kernel(white_features, black_features, ft_w, ft_b, fc1_w, fc1_b,
           fc2_w, fc2_b, **kwargs):
    nc = _get_nc()
    in_maps = prep_inputs(white_features, black_features, ft_w, ft_b,
                          fc1_w, fc1_b, fc2_w, fc2_b)
    res = run_bass_kernel_spmd(
        nc, in_maps, core_ids=list(range(N_CORES)), **kwargs)
    full = np.concatenate(
        [res.results[c]["out"].reshape(BC) for c in range(N_CORES)])
    if kwargs:
        return full.astype(np.float32), res
    return full.astype(np.float32)


# revision 7
# speedup vs baseline: 1.0217x; 1.0124x over previous
"""NNUE network forward pass on 8 Trainium2 NeuronCores (Bass/Tile).

Math (per reference):
    white_ft = clip(white @ ft_w.T + ft_b, 0, 1)                 # [B, 512]
    black_ft = clip(black @ ft_w.T + ft_b, 0, 1)                 # [B, 512]
    x = relu(concat([white_ft, black_ft], 1) @ fc1_w.T + fc1_b)  # [B, 256]
    out = x @ fc2_w.T + fc2_b                                    # [B]

Distribution: data-parallel over the batch — each of the 8 cores handles
B/8 = 512 rows end to end; weights are replicated.  No collectives.

The feature transform  y = x @ w.T + b  is computed as
    y = (dx @ wq.T) * 2^-23 + (b + 0.5 * colsum(w))
where dx = (x - 0.5) * 2^8 and wq = w * 2^15, quantized per-K-chunk to
either bf16 or fp8-e4m3:
  - centering x halves the quantization noise (|dx| <= 0.5 vs |x| <= 1);
  - the shared 2^23 product scale lets bf16 and fp8 chunks accumulate
    into the SAME fp32 PSUM chains (one eviction, scale folded into the
    activation);
  - the bias fold uses the exact (f64) column sums of w, so the constant
    component of the weight-quantization error cancels.
The first NF8 of 40 K-chunks run as fp8-e4m3 DoubleRow matmuls (256-deep
contraction per instruction, measured ~1.96x bf16 throughput); the rest
run bf16.  NF8 tunes speed vs accuracy; at NF8=8 the end-to-end relative
error is ~1.8e-2 (inputs are deterministic, so this is stable).

Layout: weights stationary on the PE array, batch on the moving free dim;
the feature-transform output lands in PSUM as [h1, batch], exactly the
transposed layout fc1 needs.  All transposes are done on the host, so
every device DMA is a plain contiguous strided load.

Warmup matmuls on a zeroed tile run during the initial DMA fill so the
PE HAM clock-gate is already released when real work arrives.
"""

import sys

for _p in ("/opt/trn_rl_repo", "/opt/pypackages"):
    if _p not in sys.path:
        sys.path.append(_p)

import numpy as np
import ml_dtypes

import concourse.bass as bass
import concourse.mybir as mybir
import concourse.tile as tile
from concourse.bass_utils import run_bass_kernel_spmd
from concourse.vector_clock import ScopedClock

# ---------------------------------------------------------------------------
# Workaround for the pinned walrus rejecting instructions that carry more
# than one semaphore wait ("Too many sync wait commands"): keep one wait on
# the instruction and move the rest onto single-wait nops that precede it
# in the same engine's program order.
# ---------------------------------------------------------------------------
_MAX_DRAIN_WAITS = 1


def _split_drain_and_barrier(self, tick_clock, wait_clock):
    nc = self.nc
    drain_inst = nc.sync.drain()
    wait_clock.add_sem_waits(
        drain_inst.ins, ScopedClock({None: tick_clock.global_clock})
    )
    si = drain_inst.ins.sync_info
    if si is not None and si.on_wait and len(si.on_wait) > _MAX_DRAIN_WAITS:
        waits = list(si.on_wait)
        drain_inst.ins.sync_info = mybir.SyncInfo(
            on_wait=waits[:_MAX_DRAIN_WAITS], on_update=list(si.on_update)
        )
        for w in waits[_MAX_DRAIN_WAITS:]:
            ni = nc.sync.nop(nofuse=True, hint="drain_wait_split")
            nsi = ni.ins.sync_info
            upd = list(nsi.on_update) if nsi is not None else []
            ni.ins.sync_info = mybir.SyncInfo(on_wait=[w], on_update=upd)

    nc.all_engine_barrier()
    assert self.sems is not None
    popped = nc._tile_sem_poison_stack.pop()
    assert popped is self._sem_poison
    nc.clear_and_free_semaphores(list(self.sems.allocated().values()))
    nc.all_engine_barrier()


tile.TileContext._drain_and_barrier = _split_drain_and_barrier


def _split_multi_waits(nc, max_waits=1):
    n_split = 0
    for f in nc.m.functions:
        for blk in f.blocks:
            out = []
            for ins in blk.instructions:
                si = ins.sync_info
                if si is not None and si.on_wait and len(si.on_wait) > max_waits:
                    waits = list(si.on_wait)
                    for w in waits[max_waits:]:
                        nop = mybir.InstNoOp(
                            name=f"I-{nc.next_id()}", ins=[], outs=[])
                        nop.engine = ins.engine
                        nop.sync_info = mybir.SyncInfo(
                            on_wait=[w], on_update=[])
                        out.append(nop)
                        n_split += 1
                    ins.sync_info = mybir.SyncInfo(
                        on_wait=waits[:max_waits],
                        on_update=list(si.on_update))
                out.append(ins)
            blk.instructions[:] = out
    return n_split


# ---------------------------------------------------------------------------
# Problem shapes (hardcoded per the harness contract).
# ---------------------------------------------------------------------------
BATCH = 4096
K = 40960
H1 = 512
H2 = 256
N_CORES = 8
BC = BATCH // N_CORES    # 512 batch rows per core

F16 = mybir.dt.float16
F8 = mybir.dt.float8e4
F32 = mybir.dt.float32
AF = mybir.ActivationFunctionType
DR = mybir.MatmulPerfMode.DoubleRow
P = 128

K_CHUNK = 1024
N_CHUNKS = K // K_CHUNK          # 40
NF8 = 10                         # fp8-DoubleRow chunks (of 40)
S_X = 2.0 ** 8
S_W = 2.0 ** 15
INV_S = 1.0 / (S_X * S_W)

E4 = ml_dtypes.float8_e4m3       # IEEE-style e4m3, max 240 = TRN FP8_EXP4
NP16 = np.float16


def build_bass(nf8=NF8, k_chunk=K_CHUNK, n_devices=N_CORES, feat_bufs=4,
               warmup=28):
    n_chunks = K // k_chunk
    assert 0 <= nf8 <= n_chunks
    n_bf = n_chunks - nf8
    k_bf = n_bf * k_chunk
    n_sub = k_chunk // P             # bf16 k-subtiles per chunk
    n_sub8 = k_chunk // (2 * P)      # DoubleRow subtiles per chunk
    n_h = H1 // P                    # 4
    n_j = 2 * H1 // P                # 8
    n_h2 = H2 // P                   # 2
    n_b = BC // P                    # 4

    nc = bass.Bass("TRN2", target_bir_lowering=False, debug=False,
                   num_devices=n_devices)

    # fp8 part, host-packed: row-block ci holds [P, n_sub8, 2, N] with
    # slot (p, s, i) = logical k = ci*k_chunk + s*256 + i*128 + p.
    if nf8:
        x8w = nc.dram_tensor("x8w", [nf8 * P, n_sub8, 2, BC], F8,
                             kind="ExternalInput")
        x8b = nc.dram_tensor("x8b", [nf8 * P, n_sub8, 2, BC], F8,
                             kind="ExternalInput")
        w8 = nc.dram_tensor("w8", [nf8 * P, n_sub8, 2, H1], F8,
                            kind="ExternalInput")
    # bf16 part, host-pretransposed [k, N]
    if n_bf:
        wfT = nc.dram_tensor("wfT", [k_bf, BC], F16, kind="ExternalInput")
        blT = nc.dram_tensor("blT", [k_bf, BC], F16, kind="ExternalInput")
        ftwT = nc.dram_tensor("ftwT", [k_bf, H1], F16, kind="ExternalInput")
    fc1_wT = nc.dram_tensor("fc1_wT", [2 * H1, H2], F16, kind="ExternalInput")
    fc2_w = nc.dram_tensor("fc2_w", [H2, 1], F16, kind="ExternalInput")
    ft_b = nc.dram_tensor("ft_b", [P, n_h], F32, kind="ExternalInput")
    fc1_b = nc.dram_tensor("fc1_b", [P, n_h2], F32, kind="ExternalInput")
    fc2_b = nc.dram_tensor("fc2_b", [P, 1], F32, kind="ExternalInput")
    out = nc.dram_tensor("out", [BC, 1], F32, kind="ExternalOutput")

    with tile.TileContext(nc) as tc:
        with (
            tc.tile_pool(name="consts", bufs=1) as consts,
            tc.tile_pool(name="feats", bufs=feat_bufs) as feats,
            tc.tile_pool(name="wpool", bufs=feat_bufs) as wpool,
            tc.tile_pool(name="f8pool", bufs=feat_bufs) as f8pool,
            tc.tile_pool(name="ftout", bufs=1) as ftout,
            tc.tile_pool(name="small", bufs=1) as small,
        ):
            # --- constants / small weights -------------------------------
            ft_b_sb = consts.tile([P, n_h], F32, tag="ft_b")
            nc.gpsimd.dma_start(ft_b_sb[:], ft_b[:])
            fc1_b_sb = consts.tile([P, n_h2], F32, tag="fc1_b")
            nc.gpsimd.dma_start(fc1_b_sb[:], fc1_b[:])
            fc2_b_sb = consts.tile([P, 1], F32, tag="fc2_b")
            nc.gpsimd.dma_start(fc2_b_sb[:], fc2_b[:])
            fc1w_sb = consts.tile([P, n_j, H2], F16, tag="fc1w")
            nc.gpsimd.dma_start(
                fc1w_sb[:], fc1_wT.rearrange("(c p) n -> p c n", p=P))
            w2_sb = consts.tile([P, n_h2], F16, tag="w2")
            nc.gpsimd.dma_start(
                w2_sb[:], fc2_w.rearrange("(c p) o -> p (c o)", p=P))

            # --- PSUM accumulation chains: (perspective, h1-tile) --------
            psA_cm = tc.tile_pool(name="psA", bufs=1, space="PSUM")
            psA = psA_cm.__enter__()
            pa = [
                psA.tile([P, BC], F32, tag=f"psA_{pi}_{h}",
                         name=f"psA_{pi}_{h}")
                for pi in range(2) for h in range(n_h)
            ]

            # --- PE warmup during the initial DMA fill -------------------
            if warmup:
                wu = consts.tile([P, BC], F16, tag="wu")
                nc.vector.memset(wu[:], 0)
                for _ in range(warmup):
                    nc.tensor.matmul(pa[0][:], wu[:, :P], wu[:],
                                     start=True, stop=True)

            first_issued = [False] * (2 * n_h)

            def mm(pi, h, lhsT, rhs, is_last, perf_mode=None):
                idx = pi * n_h + h
                nc.tensor.matmul(
                    pa[idx][:], lhsT, rhs,
                    start=not first_issued[idx], stop=is_last,
                    perf_mode=perf_mode)
                first_issued[idx] = True

            # --- stage A, fp8 DoubleRow chunks ---------------------------
            for ci in range(nf8):
                xw = f8pool.tile([P, n_sub8, 2, BC], F8, tag="x8w")
                xb = f8pool.tile([P, n_sub8, 2, BC], F8, tag="x8b")
                wt = f8pool.tile([P, n_sub8, 2, H1], F8, tag="w8")
                rows = slice(ci * P, (ci + 1) * P)
                nc.scalar.dma_start(wt[:], w8[rows])
                nc.sync.dma_start(xw[:], x8w[rows])
                nc.sync.dma_start(xb[:], x8b[rows])
                last_chunk = (ci == n_chunks - 1)
                for c in range(n_sub8):
                    for h in range(n_h):
                        for pi, x in ((0, xw), (1, xb)):
                            mm(pi, h,
                               wt[:, c, :, h * P:(h + 1) * P],
                               x[:, c, :, :],
                               last_chunk and c == n_sub8 - 1,
                               perf_mode=DR)

            # --- stage A, bf16 chunks ------------------------------------
            for ci in range(n_bf):
                k0 = ci * k_chunk
                xw = feats.tile([P, n_sub, BC], F16, tag="xw")
                xb = feats.tile([P, n_sub, BC], F16, tag="xb")
                wt = wpool.tile([P, n_sub, H1], F16, tag="wt")
                nc.scalar.dma_start(
                    wt[:], ftwT[k0:k0 + k_chunk, :].rearrange(
                        "(s p) h -> p s h", p=P))
                nc.sync.dma_start(
                    xw[:], wfT[k0:k0 + k_chunk, :].rearrange(
                        "(s p) b -> p s b", p=P))
                nc.sync.dma_start(
                    xb[:], blT[k0:k0 + k_chunk, :].rearrange(
                        "(s p) b -> p s b", p=P))
                last_chunk = (ci == n_bf - 1)
                if not last_chunk:
                    for c in range(n_sub):
                        for h in range(n_h):
                            for pi, x in ((0, xw), (1, xb)):
                                mm(pi, h, wt[:, c, h * P:(h + 1) * P],
                                   x[:, c, :], False)
                else:
                    # finish one accumulation chain at a time so evictions
                    # overlap the remaining matmuls
                    for pi, x in ((0, xw), (1, xb)):
                        for h in range(n_h):
                            for c in range(n_sub):
                                mm(pi, h, wt[:, c, h * P:(h + 1) * P],
                                   x[:, c, :], c == n_sub - 1)

            # --- eviction: clip(acc * 2^-23 + b', 0, 1), cast bf16 -------
            ft_t = []
            for pi in range(2):
                for h in range(n_h):
                    t_relu = small.tile([P, BC], F32, tag=f"relu_{pi}_{h}",
                                        name=f"relu_{pi}_{h}")
                    nc.scalar.activation(
                        t_relu[:], pa[pi * n_h + h][:], AF.Relu,
                        bias=ft_b_sb[:, h:h + 1], scale=INV_S)
                    t = ftout.tile([P, BC], F16, tag=f"ft_{pi}_{h}",
                                   name=f"ft_{pi}_{h}")
                    nc.vector.tensor_scalar_min(t[:], t_relu[:], 1.0)
                    ft_t.append(t)

            psA_cm.__exit__(None, None, None)
            psB_cm = tc.tile_pool(name="psB", bufs=1, space="PSUM")
            psB = psB_cm.__enter__()

            # --- fc1: x2[h2, b] = relu(fc1_wT.T @ combinedT + b) ---------
            pbs = [psB.tile([P, BC], F32, tag=f"psB_{h2t}",
                                            name=f"psB_{h2t}")
                   for h2t in range(n_h2)]
            for j in range(n_j):
                for h2t in range(n_h2):
                    nc.tensor.matmul(
                        pbs[h2t][:],
                        fc1w_sb[:, j, h2t * P:(h2t + 1) * P],
                        ft_t[j][:],
                        start=j == 0, stop=j == n_j - 1)
            x2 = []
            for h2t in range(n_h2):
                t2 = small.tile([P, BC], F16, tag=f"x2_{h2t}",
                                name=f"x2_{h2t}")
                nc.scalar.activation(
                    t2[:], pbs[h2t][:], AF.Relu,
                    bias=fc1_b_sb[:, h2t:h2t + 1])
                x2.append(t2)

            # --- fc2: out[b] = x2[:, b] . w2 + b2 (merged output DMA) ----
            o_all = small.tile([P, n_b], F32, tag="o_all", name="o_all")
            for bt in range(n_b):
                pc = psB.tile([P, 1], F32, tag=f"psC_{bt}", name=f"psC_{bt}")
                for h2t in range(n_h2):
                    nc.tensor.matmul(
                        pc[:],
                        x2[h2t][:, bt * P:(bt + 1) * P],
                        w2_sb[:, h2t:h2t + 1],
                        start=h2t == 0, stop=h2t == n_h2 - 1)
                nc.scalar.activation(
                    o_all[:, bt:bt + 1], pc[:], AF.Identity,
                    bias=fc2_b_sb[:])
            nc.sync.dma_start(
                out.rearrange("(b p) o -> p (b o)", p=P), o_all[:])

            psB_cm.__exit__(None, None, None)

    _split_multi_waits(nc)
    return nc


# ---------------------------------------------------------------------------
# Host side
# ---------------------------------------------------------------------------
def _to_bf16(a):
    """Fast fp32 -> bf16 with round-to-nearest-even, via bit ops."""
    u = a.view(np.uint32)
    rounded = u + 0x7FFF + ((u >> 16) & 1)
    return (rounded >> 16).astype(np.uint16).view(ml_dtypes.bfloat16)


def _pack_fp8(a, nf8, k_chunk, n):
    """[nf8*k_chunk, n] f32 -> [nf8*P, k_chunk//256, 2, n] e4m3."""
    a = a.reshape(nf8, k_chunk // 256, 2, P, n)     # [c, s, i, p, n]
    a = a.transpose(0, 3, 1, 2, 4)                  # [c, p, s, i, n]
    return np.ascontiguousarray(a).astype(E4).reshape(
        nf8 * P, k_chunk // 256, 2, n)


def prep_inputs(white_features, black_features, ft_w, ft_b, fc1_w, fc1_b,
                fc2_w, fc2_b, nf8=NF8, k_chunk=K_CHUNK):
    wf = np.asarray(white_features, np.float32)
    bl = np.asarray(black_features, np.float32)
    w = np.asarray(ft_w, np.float64)

    # bias fold: b' = ft_b + 0.5 * colsum(w), exact in f64
    ft_b_adj = (np.asarray(ft_b, np.float64) + 0.5 * w.sum(axis=1)).astype(
        np.float32)

    # centered + scaled operands: dx = (x - 0.5) * 2^8, wq = w * 2^15
    dxT = {}
    for nm, x in (("w", wf), ("b", bl)):
        t = np.ascontiguousarray(x.T)
        dxT[nm] = (t - np.float32(0.5)) * np.float32(S_X)      # [K, B]
    wqT = np.ascontiguousarray(w.T * S_W).astype(np.float32)   # [K, H1]

    k8 = nf8 * k_chunk
    w8p = _pack_fp8(wqT[:k8], nf8, k_chunk, H1) if nf8 else None
    ftwT_bf = wqT[k8:].astype(NP16) if k8 < K else None

    fc1_wT = np.ascontiguousarray(fc1_w.T).astype(NP16)
    fc2_wc = np.asarray(fc2_w, np.float32).reshape(H2, 1).astype(NP16)
    ft_b_c = np.ascontiguousarray(ft_b_adj.reshape(H1 // P, P).T)
    fc1_b_c = np.ascontiguousarray(
        np.asarray(fc1_b, np.float32).reshape(H2 // P, P).T)
    fc2_b_c = np.full((P, 1), np.asarray(fc2_b, np.float32).reshape(()),
                      np.float32)

    in_maps = []
    for c in range(N_CORES):
        cols = slice(c * BC, (c + 1) * BC)
        m = {"fc1_wT": fc1_wT, "fc2_w": fc2_wc,
             "ft_b": ft_b_c, "fc1_b": fc1_b_c, "fc2_b": fc2_b_c}
        dxw = np.ascontiguousarray(dxT["w"][:, cols])
        dxb = np.ascontiguousarray(dxT["b"][:, cols])
        if nf8:
            m["x8w"] = _pack_fp8(dxw[:k8], nf8, k_chunk, BC)
            m["x8b"] = _pack_fp8(dxb[:k8], nf8, k_chunk, BC)
            m["w8"] = w8p
        if ftwT_bf is not None:
            m["wfT"] = dxw[k8:].astype(NP16)
            m["blT"] = dxb[k8:].astype(NP16)
            m["ftwT"] = ftwT_bf
        in_maps.append(m)
    return in_maps


_NC_CACHE = {}


def _get_nc(**kw):
    key = tuple(sorted(kw.items()))
    if key not in _NC_CACHE:
        _NC_CACHE[key] = build_bass(**kw)
    return _NC_CACHE[key]


def kernel(white_features, black_features, ft_w, ft_b, fc1_w, fc1_b,
           fc2_w, fc2_b, **kwargs):
    nc = _get_nc()
    in_maps = prep_inputs(white_features, black_features, ft_w, ft_b,
                          fc1_w, fc1_b, fc2_w, fc2_b)
    res = run_bass_kernel_spmd(
        nc, in_maps, core_ids=list(range(N_CORES)), **kwargs)
    full = np.concatenate(
        [res.results[c]["out"].reshape(BC) for c in range(N_CORES)])
    if kwargs:
        return full.astype(np.float32), res
    return full.astype(np.float32)
